# revision 11
# baseline (speedup 1.0000x reference)
"""BatchHardTripletLoss on 8 Trainium2 NeuronCores (Bass/Tile), v2.

Per-core work: rows are label-sorted and columns rotated so core c's 1024 rows
sit at columns [64, 1088); every same-label (row, col) pair then falls in the
row-region cols [0, RW=1152), with tile t's groups inside the static window
[128t, 128t+WINW).

Two compute paths per core:
  * Row-region (cols [0, RW)): psum = cand = sq_j - 2 x_i.x_j via fp16 matmuls
    (sq folded with K=1 ones-matmuls). Custom DVE ops do label-masked min
    (hardest-negative part) and windowed max (hardest positive).
  * Bulk (cols [RW, 8192), no same-label pairs): TRANSPOSED tiles
    psum[j, i] = 2 x_j.x_i so that -sq_j is a per-partition bias. ACT drains
    psum -> f16 (Identity + bias) while DVE folds running per-(j%128, i)
    maxima of candN = 2x_j.x_i - sq_j; some tiles drain+fold in one fused
    DVE scalar_tensor_tensor. A PE transpose + per-tile reduce turns the
    [128, 1024] fold result into per-row bulk minima (negated space).

Tail (relu/sqrt/mean) identical to the reference, done on [128, 8] tiles.
Host combines 8 per-core (sum, count) pairs.
"""

import os
import sys

sys.path.insert(0, "/opt/trn_rl_repo")

import numpy as np

import concourse.bacc as bacc
import concourse.mybir as mybir
import concourse.tile as tile
from concourse import bass_utils

f32 = mybir.dt.float32
f16 = mybir.dt.float16
Alu = mybir.AluOpType
Act = mybir.ActivationFunctionType

BIGB = 60000.0
TAU = 1.0
MARGIN = 0.3

TRACE = False
LAST_RESULT = None

_NC_CACHE = {}
_OPS_REGISTERED = {}


def _register_ops():
    """Fused DVE ops: cand = in0 + B*[in1 == s0], reduced with MIN or MAX,
    accumulator seeded from s1."""
    if _OPS_REGISTERED:
        return _OPS_REGISTERED
    import concourse.dve_ops as dve_ops
    from concourse.dve_ops import OPS, DveOp, get_dve_sub_opcode
    from concourse.dve_spec import C0, C1, C2, Spec, Src0, Src1, eq, lower
    from concourse.dve_spec import AluOp as SAlu
    from concourse.dve_uop import DveOpSpec

    def make(name, accum_op, np_red):
        body = Src0 + eq(Src1, C0) * C2

        def ref(in0, in1, s0, s1, imm2):
            cand = (
                in0.astype(np.float32)
                + (in1.astype(np.float32) == s0) * np.float32(imm2)
            ).astype(np.float32)
            red = np_red(cand.reshape(cand.shape[0], -1), axis=-1, keepdims=True)
            seed = np.broadcast_to(np.asarray(s1, np.float32).reshape(-1, 1), red.shape)
            red = np_red(np.concatenate([red, seed], axis=1), axis=-1, keepdims=True)
            return cand, red

        spec = Spec(body=body, accum=accum_op, accum_init=C1, reference=ref)
        op = DveOp(name, spec, subdim=False, uops_sha={})
        OPS.append(op)
        dve_ops._SUB_OPCODE_FOR_NAME[name] = (
            dve_ops._CUSTOM_DVE_ROW_BASE + len(OPS) - 1
        )
        dve_ops.CUSTOM_DVE_SPECS[name] = spec
        assert dve_ops._SUB_OPCODE_FOR_NAME[name] < 0x20
        shas = {}
        for ver in ("v3", "v4"):
            try:
                dos = DveOpSpec(
                    name=name,
                    opcode=get_dve_sub_opcode(name),
                    uops=lower(spec, ver=ver),
                    rd1_en=True,
                )
                shas[ver] = dos.sha(ver)
            except Exception:
                pass
        object.__setattr__(op, "uops_sha", shas)
        return op

    _OPS_REGISTERED["min"] = make("BHTL_CAND_MIN", SAlu.MIN, np.min)
    _OPS_REGISTERED["max"] = make("BHTL_CAND_MAX", SAlu.MAX, np.max)
    return _OPS_REGISTERED


def build_nc(N, M, WINW):
    R = N // M
    T = R // 128
    RW = 128 * (T - 1) + WINW
    NB = (N - RW) // 128
    assert (N - RW) % 128 == 0

    ops = _register_ops()
    op_min, op_max = ops["min"], ops["max"]

    nc = bacc.Bacc("TRN2", target_bir_lowering=False, debug=False)

    xT_d = nc.dram_tensor("xT", [128, N], f16, kind="ExternalInput")
    xm2_d = nc.dram_tensor("xm2", [128, R], f16, kind="ExternalInput")
    xm2n_d = nc.dram_tensor("xm2n", [128, R], f16, kind="ExternalInput")
    labw_d = nc.dram_tensor("labw", [1, RW], f16, kind="ExternalInput")
    sqw_d = nc.dram_tensor("sqw", [1, RW], f16, kind="ExternalInput")
    mylab_d = nc.dram_tensor("mylab", [128, T], f32, kind="ExternalInput")
    sqi_d = nc.dram_tensor("sqi", [128, T], f32, kind="ExternalInput")
    msq_d = nc.dram_tensor("msq", [128, NB], f32, kind="ExternalInput")
    eye_d = nc.dram_tensor("eye", [128, 128], f16, kind="ExternalInput")
    out_d = nc.dram_tensor("out", [2, 1], f32, kind="ExternalOutput")

    with tile.TileContext(nc) as tc:
        with (
            tc.tile_pool(name="const", bufs=1) as cp,
            tc.tile_pool(name="convp", bufs=6) as convp,
            tc.tile_pool(name="scrp", bufs=2) as scrp,
            tc.tile_pool(name="accp", bufs=2 * T) as accp,
        ):
            # PE warmup before anything else: ramp the p-state during
            # the input-DMA wait (no input deps; PE triggers no DMAs)
            wsrc = cp.tile([128, 512], f16)
            nc.vector.memset(wsrc[:], 0.0)
            with tc.tile_pool(name="warm", bufs=1, space="PSUM") as wp:
                wps = wp.tile([128, 512], f32)
                for _ in range(12):
                    nc.tensor.matmul(
                        wps[:], wsrc[:, 0:128], wsrc[:], start=True, stop=True
                    )

            xT = cp.tile([128, N], f16)
            xm2n = cp.tile([128, R], f16)
            xm2 = cp.tile([128, R], f16)
            labw_b = cp.tile([128, RW], f16)
            sqw_t = cp.tile([1, RW], f16)
            mylab = cp.tile([128, T], f32)
            sqi = cp.tile([128, T], f32)
            msq = cp.tile([128, NB], f32)
            eye = cp.tile([128, 128], f16)

            def xchunk(q):
                return (xT[:, q * 1024 : (q + 1) * 1024],
                        xT_d.ap()[:, q * 1024 : (q + 1) * 1024])

            # sync queue: first row-region xT chunk, then bulk-critical consts
            nc.sync.dma_start(xT[:, 0:512], xT_d.ap()[:, 0:512])
            nc.sync.dma_start(xT[:, 512:1024], xT_d.ap()[:, 512:1024])
            nc.sync.dma_start(msq[:], msq_d.ap())
            nc.sync.dma_start(*xchunk(3))
            nc.sync.dma_start(*xchunk(6))
            nc.sync.dma_start(sqi[:], sqi_d.ap())
            # gpsimd queue: row stationaries + second chunk
            nc.gpsimd.dma_start(xm2n[:], xm2n_d.ap())
            nc.gpsimd.dma_start(sqw_t[:], sqw_d.ap())
            nc.gpsimd.dma_start(mylab[:], mylab_d.ap())
            nc.gpsimd.dma_start(*xchunk(1))
            nc.gpsimd.dma_start(*xchunk(4))
            nc.gpsimd.dma_start(*xchunk(7))
            nc.gpsimd.dma_start(eye[:], eye_d.ap())
            # scalar queue: labels bcast, bulk moving, remaining chunks
            nc.scalar.dma_start(labw_b[:], labw_d.ap().broadcast_to([128, RW]))
            nc.scalar.dma_start(xm2[:], xm2_d.ap())
            nc.scalar.dma_start(*xchunk(2))
            nc.scalar.dma_start(*xchunk(5))

            ones_row = cp.tile([1, 128], f16)
            nc.vector.memset(ones_row[:], 1.0)
            wact = cp.tile([128, 512], f16)
            wbias = cp.tile([128, 1], f32)
            nc.vector.memset(wbias[:], 0.0)
            nc.scalar.activation(wact[:], wsrc[:], Act.Identity, bias=wbias[:], scale=1.0)
            nc.scalar.activation(wact[:, 0:8], wsrc[:, 0:8], Act.Sqrt)


            negwin = cp.tile([128, T], f32)
            poswin = cp.tile([128, T], f32)
            negbTn = cp.tile([128, T], f32)

            # fold chains: 4 per half x 2 ping-pong acc buffers
            accs = [
                [
                    cp.tile([128, R], f16, name=f"acc_{ch}_{i}")
                    for i in range(2)
                ]
                for ch in range(4)
            ]
            cur = [0] * 4
            seen = [False] * 4

            pools = {}

            def bulk(jt):
                bc = RW + 128 * jt
                pp = pools["pp"]
                ps = pp.tile([128, 1024], f32, tag="ps")
                nc.tensor.matmul(
                    ps[:, 0:512], xT[:, bc : bc + 128], xm2[:, 0:512],
                    start=True, stop=True,
                )
                nc.tensor.matmul(
                    ps[:, 512:1024], xT[:, bc : bc + 128], xm2[:, 512:1024],
                    start=True, stop=True,
                )
                ch = jt % 4
                dve_route = (jt % 16) == 7
                A = accs[ch][cur[ch]]
                B = accs[ch][1 - cur[ch]]
                mb = msq[:, jt : jt + 1]
                if not seen[ch]:
                    if dve_route:
                        nc.vector.tensor_scalar_add(A[:], ps[:], mb)
                    else:
                        nc.scalar.activation(
                            A[:], ps[:], Act.Identity, bias=mb, scale=1.0
                        )
                    seen[ch] = True
                elif dve_route:
                    nc.vector.scalar_tensor_tensor(
                        B[:], ps[:], mb, A[:], op0=Alu.add, op1=Alu.max
                    )
                    cur[ch] = 1 - cur[ch]
                else:
                    conv = convp.tile([128, R], f16, tag="conv")
                    nc.scalar.activation(
                        conv[:], ps[:], Act.Identity, bias=mb, scale=1.0
                    )
                    nc.vector.tensor_tensor(B[:], A[:], conv[:], Alu.max)
                    cur[ch] = 1 - cur[ch]

            def row(t):
                pp = pools["pp"]
                segs = [(0, 1024), (1024, RW)]
                pstiles = []
                for s0, s1 in segs:
                    ps = pp.tile([128, 1024], f32, tag="ps")
                    for m0 in range(s0, s1, 512):
                        m1 = min(m0 + 512, s1)
                        nc.tensor.matmul(
                            ps[:, m0 - s0 : m1 - s0],
                            xm2n[:, t * 128 : (t + 1) * 128],
                            xT[:, m0:m1], start=True, stop=False,
                        )
                        nc.tensor.matmul(
                            ps[:, m0 - s0 : m1 - s0], ones_row[:],
                            sqw_t[:, m0:m1], start=False, stop=True,
                        )
                    pstiles.append(ps)
                accn = None
                for (s0, s1), ps in zip(segs, pstiles):
                    w = s1 - s0
                    scrm = scrp.tile([128, 1024], f16, tag="scrm")
                    am = accp.tile([128, 1], f32, tag="am")
                    nc.vector._custom_dve(
                        op_min, out=scrm[:, 0:w], in0=ps[:, 0:w],
                        in1=labw_b[:, s0:s1],
                        s0=mylab[:, t : t + 1],
                        s1=(1e30 if accn is None else accn[:]), imm2=BIGB,
                        accum_out=am[:],
                    )
                    accn = am
                nc.vector.tensor_copy(negwin[:, t : t + 1], accn[:])
                w0 = 128 * t
                accx = None
                for (s0, s1), ps in zip(segs, pstiles):
                    lo = max(w0, s0)
                    hi = min(w0 + WINW, s1)
                    if lo >= hi:
                        continue
                    scrx = scrp.tile([128, WINW], f16, tag="scrx")
                    ax = accp.tile([128, 1], f32, tag="ax")
                    nc.vector._custom_dve(
                        op_max, out=scrx[:, 0 : hi - lo],
                        in0=ps[:, lo - s0 : hi - s0],
                        in1=labw_b[:, lo:hi],
                        s0=mylab[:, t : t + 1],
                        s1=(-1e30 if accx is None else accx[:]), imm2=BIGB,
                        accum_out=ax[:],
                    )
                    accx = ax
                nc.vector.tensor_copy(poswin[:, t : t + 1], accx[:])

            # interleave bulk tiles and row tiles
            with tc.tile_pool(name="pp", bufs=4, space="PSUM") as pp_:
                pools["pp"] = pp_
                row(0)
                t_next = 1
                for jt in range(NB):
                    bulk(jt)
                    if jt % 7 == 6 and t_next < T:
                        row(t_next)
                        t_next += 1
                while t_next < T:
                    row(t_next)
                    t_next += 1

            f0 = accs[0][cur[0]]
            f1 = accs[1][cur[1]]
            f2 = accs[2][cur[2]]
            f3 = accs[3][cur[3]]
            nc.vector.tensor_tensor(f0[:], f0[:], f1[:], Alu.max)
            nc.vector.tensor_tensor(f2[:], f2[:], f3[:], Alu.max)
            nc.vector.tensor_tensor(f0[:], f0[:], f2[:], Alu.max)
            with tc.tile_pool(name="tp", bufs=1, space="PSUM") as tpp:
                psT = tpp.tile([128, R], f16)
                for k in range(T):
                    nc.tensor.transpose(
                        psT[:, k * 128 : (k + 1) * 128],
                        f0[:, k * 128 : (k + 1) * 128],
                        eye[:],
                    )
                nc.vector.tensor_reduce(
                    negbTn[:],
                    psT[:].rearrange("p (k w) -> p k w", w=128),
                    axis=mybir.AxisListType.X, op=Alu.max,
                )

                # tail: per-row loss on [128, T]
                hncand = cp.tile([128, T], f32)
                nc.vector.scalar_tensor_tensor(
                    hncand[:], negbTn[:], -1.0, negwin[:],
                    op0=Alu.mult, op1=Alu.min,
                )

                hn2 = cp.tile([128, T], f32)
                nc.vector.tensor_add(hn2[:], hncand[:], sqi[:])
                hp2 = cp.tile([128, T], f32)
                nc.vector.scalar_tensor_tensor(
                    hp2[:], poswin[:], -BIGB, sqi[:], op0=Alu.add, op1=Alu.add
                )

                vp = cp.tile([128, T], f32)
                nc.vector.tensor_single_scalar(vp[:], hp2[:], TAU, Alu.is_gt)
                vn = cp.tile([128, T], f32)
                nc.vector.tensor_single_scalar(vn[:], hn2[:], BIGB / 2.0, Alu.is_lt)
                valid = cp.tile([128, T], f32)
                nc.vector.tensor_mul(valid[:], vp[:], vn[:])

                hp2c = cp.tile([128, T], f32)
                nc.vector.tensor_scalar_max(hp2c[:], hp2[:], 0.0)
                hn2c = cp.tile([128, T], f32)
                nc.vector.tensor_scalar_max(hn2c[:], hn2[:], 0.0)
                hp = cp.tile([128, T], f32)
                nc.scalar.activation(hp[:], hp2c[:], Act.Sqrt)
                hn = cp.tile([128, T], f32)
                nc.scalar.activation(hn[:], hn2c[:], Act.Sqrt)

                d = cp.tile([128, T], f32)
                nc.vector.scalar_tensor_tensor(
                    d[:], hp[:], MARGIN, hn[:], op0=Alu.add, op1=Alu.subtract
                )
                relu_d = cp.tile([128, T], f32)
                nc.vector.tensor_scalar_max(relu_d[:], d[:], 0.0)
                pr = cp.tile([128, T], f32)
                nc.vector.tensor_mul(pr[:], relu_d[:], valid[:])

                stack = cp.tile([128, 2], f32)
                nc.vector.tensor_reduce(
                    stack[:, 0:1], pr[:], axis=mybir.AxisListType.X, op=Alu.add
                )
                nc.vector.tensor_reduce(
                    stack[:, 1:2], valid[:], axis=mybir.AxisListType.X, op=Alu.add
                )
                ones_col32 = cp.tile([128, 1], f32)
                nc.vector.memset(ones_col32[:], 1.0)
                pt = tpp.tile([2, 1], f32)
                nc.tensor.matmul(pt[:], stack[:], ones_col32[:], start=True, stop=True)
                outsb = cp.tile([2, 1], f32)
                nc.scalar.copy(outsb[:], pt[:])
                nc.sync.dma_start(out_d.ap(), outsb[:])

    nc.compile()
    return nc


def _prep_inputs(x, labels, M, WINW):
    N, D = x.shape
    R = N // M
    T = R // 128
    RW = 128 * (T - 1) + WINW
    NB = (N - RW) // 128

    labels = np.asarray(labels)
    perm = np.argsort(labels, kind="stable")
    lab_s = labels[perm]
    x16 = np.asarray(x, np.float32)[perm].astype(np.float16)
    xsT = np.ascontiguousarray(x16.T)  # [128, N]
    sq = (x16.astype(np.float32) ** 2).sum(axis=1)  # [N] f32

    first = np.zeros(N, dtype=np.int64)
    last = np.zeros(N, dtype=np.int64)
    start = 0
    for i in range(1, N + 1):
        if i == N or lab_s[i] != lab_s[start]:
            first[start:i] = start
            last[start:i] = i - 1
            start = i

    # window check: per row r of core c at tile t, group must sit in
    # [128t, 128t + WINW) of that core's rotated column space
    ri = np.arange(N)
    c_of = ri // R
    t_of = (ri % R) // 128
    lo = first - c_of * R + 64
    hi = last - c_of * R + 64
    ok = np.all((lo >= 128 * t_of) & (hi < 128 * t_of + WINW))
    if not ok:
        return None

    eye = np.eye(128, dtype=np.float16)
    in_maps = []
    for c in range(M):
        colmap = (np.arange(N) + c * R - 64) % N
        rows = slice(c * R, (c + 1) * R)
        xm2 = np.ascontiguousarray(2.0 * xsT[:, rows]).astype(np.float16)
        in_maps.append(
            {
                "xT": np.ascontiguousarray(xsT[:, colmap]),
                "xm2": xm2,
                "xm2n": np.ascontiguousarray(-xm2),
                "labw": np.ascontiguousarray(
                    lab_s[colmap[:RW]].astype(np.float16).reshape(1, RW)
                ),
                "sqw": np.ascontiguousarray(
                    sq[colmap[:RW]].astype(np.float16).reshape(1, RW)
                ),
                "mylab": np.ascontiguousarray(
                    lab_s[rows].astype(np.float32).reshape(T, 128).T
                ),
                "sqi": np.ascontiguousarray(
                    sq[rows].astype(np.float32).reshape(T, 128).T
                ),
                "msq": np.ascontiguousarray(
                    (-sq[colmap[RW : RW + 128 * NB]])
                    .astype(np.float32)
                    .reshape(NB, 128)
                    .T
                ),
                "eye": eye,
            }
        )
    return in_maps


def kernel(embeddings, labels):
    global LAST_RESULT
    x = np.asarray(embeddings, dtype=np.float32)
    lab = np.asarray(labels)
    N, D = x.shape
    M = 8
    assert D == 128 and N % (M * 128) == 0

    WINW = 256
    in_maps = _prep_inputs(x, lab, M, WINW)
    while in_maps is None and WINW < N // M:
        WINW *= 2
        in_maps = _prep_inputs(x, lab, M, WINW)
    assert in_maps is not None

    key = (N, M, WINW)
    if key not in _NC_CACHE:
        _NC_CACHE[key] = build_nc(N, M, WINW)
    nc = _NC_CACHE[key]

    if TRACE:
        _install_ntff_hook()
    res = bass_utils.run_bass_kernel_spmd(
        nc, in_maps, core_ids=list(range(M)), trace=TRACE
    )
    LAST_RESULT = res

    total = 0.0
    cnt = 0.0
    for c in range(M):
        o = res.results[c]["out"]
        total += float(o[0, 0])
        cnt += float(o[1, 0])
    loss = total / max(cnt, 1.0) if cnt > 0 else 0.0
    return np.float32(loss)


def _install_ntff_hook():
    """The container's antenv stub lacks axon_hooks; provide it so
    run_bass_kernel_spmd(trace=True) can capture NTFF profiles."""
    import contextlib
    import ctypes
    import types

    try:
        from antenv.axon_hooks import get_axon_ntff_profile_hook  # noqa: F401

        return
    except ImportError:
        pass
    import antenv

    mod = types.ModuleType("antenv.axon_hooks")
    _h = {"h": None}
    mod.set_axon_ntff_profile_hook = lambda h: _h.__setitem__("h", h)
    mod.get_axon_ntff_profile_hook = lambda: _h["h"]
    sys.modules["antenv.axon_hooks"] = mod
    antenv.axon_hooks = mod

    so_path = "/opt/axon/libaxon_pjrt.so"
    if not os.path.exists(so_path):
        return
    lib = ctypes.CDLL(so_path)
    if not hasattr(lib, "axon_start_nrt_profile"):
        return
    lib.axon_start_nrt_profile.argtypes = [
        ctypes.POINTER(ctypes.c_int64),
        ctypes.c_size_t,
    ]
    lib.axon_start_nrt_profile.restype = ctypes.c_int64
    lib.axon_stop_nrt_profile.argtypes = [ctypes.c_char_p]
    lib.axon_stop_nrt_profile.restype = ctypes.c_int64

    @contextlib.contextmanager
    def _hook(output_dir, device_ids):
        import jax

        jax.devices()
        if device_ids:
            ids = (ctypes.c_int64 * len(device_ids))(*device_ids)
            rc = lib.axon_start_nrt_profile(ids, len(device_ids))
        else:
            rc = lib.axon_start_nrt_profile(None, 0)
        if rc != 0:
            raise RuntimeError(f"axon_start_nrt_profile rc={rc}")
        try:
            yield
        finally:
            n = lib.axon_stop_nrt_profile(str(output_dir).encode())
            print(f"profile: {n} file(s) written to {output_dir}", file=sys.stderr)

    mod.set_axon_ntff_profile_hook(_hook)
